# revision 1
# baseline (speedup 1.0000x reference)
"""CrossPhaseRoutingLayer Trainium2 kernel.

Full inputs -> full output. Data-parallel over the fused B*C=512 sequence axis
across 8 NeuronCores (64 sequences each). Per core, sequences are processed in
groups of G=4 (T = G*96 = 384 token columns per group).

Algebraic restructuring (host-side, weight-only folds, validated to ~6e-6):
  - Sender attention q = router @ Wq_s + bq_s is input-independent, so sender
    scores fold into one matrix: scores^T = M_score^T @ x^T + c_score, where
    M_score[d,(h,r)] = Wk_s[d,h-slice] . q_s[r,h-slice] / sqrt(E).
  - The sender value/output path runs in "mix first, project later" order:
    T_mix = A1 @ x (per head/router), then per-head Wv_s slice, then Wo_s.
    Sender biases collapse: c_send = bv_s @ Wo_s + bo_s.
  - Receiver: scale folds into Wq_r/bq_r; bv_r folds into c_recv = bv_r@Wo_r+bo_r.
  - Softmaxes skip max-subtraction (scores provably tiny: |s| < 0.1).

On-chip layout: activations live transposed (x^T: [D-chunk(128 part), token])
for all D-contraction matmuls; token-partition tiles where per-token free-dim
reductions (softmax) or token-contraction (A1 @ x) are needed; PE transposes
convert. Big matmuls (N>=256) run as float32r (~2.7x fp32 rate, rel err ~1e-4).
LayerNorm over the partitioned D axis uses ones-matmul reductions and a
[K=1] broadcast matmul.
"""
import numpy as np

import concourse.bacc as bacc
import concourse.bass as bass
import concourse.mybir as mybir
import concourse.tile as tile
from concourse.bass_utils import run_bass_kernel_spmd
from concourse.masks import make_identity

FP = mybir.dt.float32
FPR = mybir.dt.float32r
AX = mybir.AxisListType
OP = mybir.AluOpType
ACTF = mybir.ActivationFunctionType

B, C, L, D = 16, 32, 96, 512
R, H = 8, 4
E = D // H            # 128
HR = H * R            # 32
DC = D // 128         # 4 D-chunks
OC = (4 * D) // 128   # 16 MLP hidden chunks
EPS = 1e-5
N_CORES = 8
G = 4                 # sequences per group
T = G * L             # 384 token columns per group

W_NAMES = ["Msc", "Wv_s", "Wo_s", "Wq_r", "Wk_r", "Wv_r", "Wo_r", "W1", "W2"]
V_NAMES = ["c_score", "c_send", "c_recv", "bq_r", "bk_r", "b1", "b2",
           "ln1_g", "ln1_b", "ln2_g", "ln2_b"]


def build_core_kernel(n_seq: int):
    """Bass program for one core processing n_seq sequences."""
    assert n_seq % G == 0
    n_groups = n_seq // G
    nc = bacc.Bacc(None)

    z = nc.declare_dram_parameter("z", [n_seq * L, D], FPR, isOutput=False)
    out = nc.declare_dram_parameter("out", [n_seq * L, D], FP, isOutput=True)
    wd = {}
    for name, shape in [("Msc", [D, HR]), ("Wv_s", [D, D]), ("Wo_s", [D, D]),
                        ("Wq_r", [D, D]), ("Wk_r", [D, D]), ("Wv_r", [D, D]),
                        ("Wo_r", [D, D]), ("W1", [D, 4 * D]), ("W2", [4 * D, D])]:
        wd[name] = nc.declare_dram_parameter(name, shape, FPR, isOutput=False)
    for name, n in [("c_score", HR), ("c_send", D), ("c_recv", D), ("bq_r", D),
                    ("bk_r", D), ("b1", 4 * D), ("b2", D), ("ln1_g", D),
                    ("ln1_b", D), ("ln2_g", D), ("ln2_b", D)]:
        wd[name] = nc.declare_dram_parameter(name, [n], FP, isOutput=False)

    with tile.TileContext(nc) as tc:
        with tc.tile_pool(name="wpool", bufs=1) as wp, \
             tc.tile_pool(name="xin", bufs=2) as px, \
             tc.tile_pool(name="act1", bufs=1) as pa, \
             tc.tile_pool(name="sm", bufs=2) as psm, \
             tc.tile_pool(name="micro", bufs=3) as pmi, \
             tc.tile_pool(name="big3", bufs=3) as pb3, \
             tc.tile_pool(name="otok", bufs=1) as po, \
             tc.tile_pool(name="ps", bufs=1, space="PSUM") as ps:

            # ---------------- resident weights / constants -----------------
            w = {}
            w["Msc"] = wp.tile([128, DC, HR], FPR, name="w_Msc")
            for name in ["Wv_s", "Wo_s", "Wq_r", "Wk_r", "Wv_r", "Wo_r"]:
                w[name] = wp.tile([128, DC, D], FPR, name=f"w_{name}")
            w["W1"] = wp.tile([128, DC, 4 * D], FPR, name="w_W1")
            w["W2"] = wp.tile([128, OC, D], FPR, name="w_W2")
            for name in W_NAMES:
                nc.sync.dma_start(
                    out=w[name],
                    in_=wd[name].rearrange("(c p) x -> p c x", p=128))
            w["c_score"] = wp.tile([HR, 1], FP, name="w_c_score")
            nc.sync.dma_start(out=w["c_score"],
                              in_=wd["c_score"].rearrange("(p o) -> p o", o=1))
            for name in ["c_send", "c_recv", "bq_r", "bk_r", "b2",
                         "ln1_g", "ln1_b", "ln2_g", "ln2_b"]:
                w[name] = wp.tile([128, DC], FP, name=f"w_{name}")
                nc.sync.dma_start(out=w[name],
                                  in_=wd[name].rearrange("(c p) -> p c", p=128))
            w["b1"] = wp.tile([128, OC], FP, name="w_b1")
            nc.sync.dma_start(out=w["b1"],
                              in_=wd["b1"].rearrange("(c p) -> p c", p=128))

            ident = wp.tile([128, 128], FP, name="ident")
            make_identity(nc, ident)
            identr = wp.tile([128, 128], FPR, name="identr")
            nc.scalar.copy(out=identr, in_=ident)
            ones_f = wp.tile([128, 1], FP, name="ones_f")
            nc.vector.memset(ones_f, 1.0)
            ones_r = wp.tile([128, 1], FPR, name="ones_r")
            nc.scalar.copy(out=ones_r, in_=ones_f)
            eps_t = wp.tile([1, 1], FP, name="eps_t")
            nc.vector.memset(eps_t, EPS)

            for gi in range(n_groups):
                group_body(nc, tc, w, ident, identr, ones_r, eps_t,
                           z, out, gi,
                           px, pa, psm, pmi, pb3, po, ps)
    nc.finalize()
    return nc


def layernorm_T(nc, w, ones_r, eps_t, pmi, pb3, ps,
                s_T, out_tile, g_name, b_name, out_dtype, tag):
    """LN over the partition-split D axis of s_T [128, DC, T] -> out_tile."""
    mean_ps = ps.tile([1, T], FP, name=f"mean_ps{tag}", tag="big", bufs=2)
    for k in range(DC):
        nc.tensor.matmul(out=mean_ps, lhsT=ones_r, rhs=s_T[:, k, :],
                         start=(k == 0), stop=(k == DC - 1))
    msc = pmi.tile([1, T], FP, name=f"msc{tag}", tag="micro")
    nc.scalar.activation(out=msc, in_=mean_ps, func=ACTF.Copy, scale=1.0 / D)

    ss_ps = ps.tile([1, T], FP, name=f"ss_ps{tag}", tag="big", bufs=2)
    for k in range(DC):
        sq = pb3.tile([128, T], FPR, name=f"sq{tag}", tag="sq", bufs=2)
        nc.vector.tensor_mul(out=sq, in0=s_T[:, k, :].bitcast(FP),
                             in1=s_T[:, k, :].bitcast(FP))
        nc.tensor.matmul(out=ss_ps, lhsT=ones_r, rhs=sq,
                         start=(k == 0), stop=(k == DC - 1))

    msc2 = pmi.tile([1, T], FP, name=f"msc2{tag}", tag="micro")
    nc.vector.tensor_mul(out=msc2, in0=msc, in1=msc)
    var_s = pmi.tile([1, T], FP, name=f"var{tag}", tag="micro")
    nc.vector.scalar_tensor_tensor(out=var_s, in0=ss_ps, scalar=1.0 / D,
                                   in1=msc2, op0=OP.mult, op1=OP.subtract)
    srt = pmi.tile([1, T], FP, name=f"srt{tag}", tag="micro")
    nc.scalar.activation(out=srt, in_=var_s, func=ACTF.Sqrt, bias=eps_t)
    rstd = pmi.tile([1, T], FP, name=f"rstd{tag}", tag="micro")
    nc.vector.reciprocal(out=rstd, in_=srt)
    mr = pmi.tile([1, T], FP, name=f"mr{tag}", tag="micro")
    nc.vector.tensor_mul(out=mr, in0=msc, in1=rstd)

    rstdB = pb3.tile([128, T], FP, name=f"rstdB{tag}", tag="rstdB", bufs=2)
    nc.gpsimd.partition_broadcast(rstdB, rstd)
    mrB = pb3.tile([128, T], FP, name=f"mrB{tag}", tag="mrB", bufs=2)
    nc.gpsimd.partition_broadcast(mrB, mr)

    for k in range(DC):
        t1 = pb3.tile([128, T], FP, name=f"t1{tag}", tag="lnt", bufs=2)
        nc.vector.tensor_mul(out=t1, in0=s_T[:, k, :].bitcast(FP), in1=rstdB)
        nc.vector.tensor_sub(out=t1, in0=t1, in1=mrB)
        nc.vector.tensor_scalar(out=out_tile[:, k, :],
                                in0=t1,
                                scalar1=w[g_name][:, k:k + 1], op0=OP.mult,
                                scalar2=w[b_name][:, k:k + 1], op1=OP.add)


def group_body(nc, tc, w, ident, identr, ones_r, eps_t, z, out, gi,
               px, pa, psm, pmi, pb3, po, ps):
    r0 = gi * T   # first DRAM row of the group

    # ---- load x (token-partition) and build x^T ----
    x_tok = px.tile([L, G, D], FPR, name="x_tok")
    nc.sync.dma_start(out=x_tok,
                      in_=z[r0:r0 + T, :].rearrange("(g l) d -> l g d", g=G))
    xT = pa.tile([128, DC, T], FPR, name="xT")
    for g in range(G):
        for dc in range(DC):
            pt = ps.tile([128, L], FPR, name="pt_x", tag="sp", bufs=2)
            nc.tensor.transpose(out=pt, in_=x_tok[:, g, dc * 128:(dc + 1) * 128],
                                identity=identr[:L, :L])
            nc.scalar.copy(out=xT[:, dc, g * L:(g + 1) * L], in_=pt)
    xTr = xT  # FPR view; read with .bitcast(FP)

    # ---- sender scores^T [HR, T] and softmax over tokens ----
    sc_ps = ps.tile([HR, T], FP, name="sc_ps", tag="big", bufs=2)
    for k in range(DC):
        nc.tensor.matmul(out=sc_ps, lhsT=w["Msc"][:, k, :], rhs=xTr[:, k, :],
                         start=(k == 0), stop=(k == DC - 1))
    e1 = psm.tile([HR, T], FP, name="e1")
    nc.scalar.activation(out=e1, in_=sc_ps, func=ACTF.Exp, bias=w["c_score"])
    s1sum = psm.tile([HR, G], FP, name="s1sum")
    nc.vector.tensor_reduce(out=s1sum, in_=e1.rearrange("p (g l) -> p g l", g=G),
                            axis=AX.X, op=OP.add)
    r1 = psm.tile([HR, G], FP, name="r1")
    nc.vector.reciprocal(out=r1, in_=s1sum)

    # A1^T per sequence (token-partition), un/normalized handling:
    # normalize in [HR, L] layout then transpose to [L, HR].
    a1t = []
    for g in range(G):
        a1n = psm.tile([HR, L], FP, name=f"a1n{g}", tag="a1n", bufs=2)
        nc.vector.tensor_scalar_mul(out=a1n, in0=e1[:, g * L:(g + 1) * L],
                                    scalar1=r1[:, g:g + 1])
        a1p = ps.tile([L, HR], FP, name="a1p", tag="sp", bufs=2)
        nc.tensor.transpose(out=a1p, in_=a1n, identity=ident[:HR, :HR])
        a1s = psm.tile([L, HR], FPR, name=f"a1s{g}", tag="a1s", bufs=4)
        nc.scalar.copy(out=a1s, in_=a1p)
        a1t.append(a1s)

    # ---- T_mix^T [(dc), (g, hr)] = x_chunk.T @ A1^T  (contract tokens) ----
    tm_ps = ps.tile([128, DC, G, HR], FP, name="tm_ps", tag="sp", bufs=2)
    for g in range(G):
        for dc in range(DC):
            nc.tensor.matmul(out=tm_ps[:, dc, g, :],
                             lhsT=x_tok[:, g, dc * 128:(dc + 1) * 128],
                             rhs=a1t[g], start=True, stop=True)
    TmT = pa.tile([128, DC, G, HR], FPR, name="TmT")
    nc.scalar.copy(out=TmT, in_=tm_ps)

    # ---- out_cat^T chunk h = Wv_s_h^T @ Tm_h^T   [128,(g,r)] ----
    oc_ps = ps.tile([128, H, G, R], FP, name="oc_ps", tag="sp", bufs=2)
    for h in range(H):
        for k in range(DC):
            nc.tensor.matmul(out=oc_ps[:, h, :, :],
                             lhsT=w["Wv_s"][:, k, h * E:(h + 1) * E],
                             rhs=TmT[:, k, :, h * R:(h + 1) * R],
                             start=(k == 0), stop=(k == DC - 1))
    Oc = pa.tile([128, H, G, R], FPR, name="Oc")
    nc.scalar.copy(out=Oc, in_=oc_ps)

    # ---- router_buffer^T [(dc), (g, r)] = Wo_s^T @ out_cat^T + c_send ----
    rb_ps = ps.tile([128, DC, G, R], FP, name="rb_ps", tag="sp", bufs=2)
    for dc in range(DC):
        for k in range(DC):
            nc.tensor.matmul(out=rb_ps[:, dc, :, :],
                             lhsT=w["Wo_s"][:, k, dc * 128:(dc + 1) * 128],
                             rhs=Oc[:, k, :, :],
                             start=(k == 0), stop=(k == DC - 1))
    rb = pa.tile([128, DC, G, R], FPR, name="rb")
    for dc in range(DC):
        nc.scalar.activation(out=rb[:, dc, :, :],
                             in_=rb_ps[:, dc, :, :], func=ACTF.Identity,
                             bias=w["c_send"][:, dc:dc + 1])

    # ---- receiver k^T [(dc=head), (g,r)] ----
    kt_ps = ps.tile([128, DC, G, R], FP, name="kt_ps", tag="sp", bufs=2)
    for dc in range(DC):
        for k in range(DC):
            nc.tensor.matmul(out=kt_ps[:, dc, :, :],
                             lhsT=w["Wk_r"][:, k, dc * 128:(dc + 1) * 128],
                             rhs=rb[:, k, :, :],
                             start=(k == 0), stop=(k == DC - 1))
    kT = pa.tile([128, DC, G, R], FPR, name="kT")
    for dc in range(DC):
        nc.scalar.activation(out=kT[:, dc, :, :],
                             in_=kt_ps[:, dc, :, :], func=ACTF.Identity,
                             bias=w["bk_r"][:, dc:dc + 1])

    # ---- receiver v in router-partition layout [8, D] per seq (bias folded) ----
    v_sb = []
    for g in range(G):
        v_ps = ps.tile([R, D], FP, name="v_ps", tag="sp", bufs=2)
        for k in range(DC):
            nc.tensor.matmul(out=v_ps, lhsT=rb[:, k, g, :],
                             rhs=w["Wv_r"][:, k, :],
                             start=(k == 0), stop=(k == DC - 1))
        v_g = psm.tile([R, D], FPR, name=f"v_g{g}", tag="v_g", bufs=4)
        nc.scalar.copy(out=v_g, in_=v_ps)
        v_sb.append(v_g)

    # ---- receiver q^T [(dc), T] (scale+bias pre-folded) ----
    qT = pa.tile([128, DC, T], FPR, name="qT", tag="big_a")
    for dc in range(DC):
        q_ps = ps.tile([128, T], FP, name="q_ps", tag="big", bufs=2)
        for k in range(DC):
            nc.tensor.matmul(out=q_ps, lhsT=w["Wq_r"][:, k, dc * 128:(dc + 1) * 128],
                             rhs=xTr[:, k, :], start=(k == 0), stop=(k == DC - 1))
        nc.scalar.activation(out=qT[:, dc, :], in_=q_ps,
                             func=ACTF.Identity, bias=w["bq_r"][:, dc:dc + 1])

    # ---- receiver scores -> softmax -> mix (dual-layout, no transposes) ----
    aT = pa.tile([128, DC, T], FPR, name="aT", tag="big_b")
    for g in range(G):
        # token-partition scores for the softmax denominators
        s2_ps = ps.tile([L, H, R], FP, name="s2_ps", tag="sp", bufs=2)
        for h in range(H):
            nc.tensor.matmul(out=s2_ps[:, h, :],
                             lhsT=qT[:, h, g * L:(g + 1) * L],
                             rhs=kT[:, h, g, :], start=True, stop=True)
        e2 = psm.tile([L, H, R], FP, name=f"e2{g}", tag="e2", bufs=2)
        nc.scalar.activation(out=e2, in_=s2_ps, func=ACTF.Exp)
        ssum = psm.tile([L, H], FP, name=f"ssum{g}", tag="ssum", bufs=2)
        nc.vector.tensor_reduce(out=ssum, in_=e2, axis=AX.X, op=OP.add)
        r2 = psm.tile([L, H], FP, name=f"r2{g}", tag="r2", bufs=2)
        nc.vector.reciprocal(out=r2, in_=ssum)
        for h in range(H):
            # r2 column h -> [1, L] at partition 0, broadcast on gpsimd
            r2p = ps.tile([1, L], FP, name="r2p", tag="sp", bufs=2)
            nc.tensor.transpose(out=r2p, in_=r2[:, h:h + 1], identity=ident[:L, :L])
            r2T = psm.tile([1, L], FP, name=f"r2T{g}{h}", tag="r2T", bufs=2)
            nc.scalar.copy(out=r2T, in_=r2p)
            # router-partition scores -> exp directly (same math, swapped operands)
            s2t_ps = ps.tile([R, L], FP, name="s2t_ps", tag="sp", bufs=2)
            nc.tensor.matmul(out=s2t_ps, lhsT=kT[:, h, g, :],
                             rhs=qT[:, h, g * L:(g + 1) * L], start=True, stop=True)
            e2t = psm.tile([R, L], FPR, name=f"e2t{g}{h}", tag="e2t", bufs=2)
            nc.scalar.activation(out=e2t, in_=s2t_ps, func=ACTF.Exp)
            r2B = pb3.tile([128, L], FP, name=f"r2B{g}{h}", tag="r2B", bufs=2)
            nc.gpsimd.partition_broadcast(r2B, r2T)
            # mix: apT chunk (head h) = v_h^T-as-lhsT @ e2t, normalize on copy-out
            apT_ps = ps.tile([128, L], FP, name="apT_ps", tag="sp", bufs=2)
            nc.tensor.matmul(out=apT_ps, lhsT=v_sb[g][:, h * E:(h + 1) * E],
                             rhs=e2t, start=True, stop=True)
            nc.vector.tensor_mul(out=aT[:, h, g * L:(g + 1) * L],
                                 in0=apT_ps, in1=r2B)

    # ---- attn2^T = Wo_r^T @ attn_pre^T + c_recv; residual; LN1 ----
    s1T = pa.tile([128, DC, T], FPR, name="s1T", tag="big_a")
    for dc in range(DC):
        at2_ps = ps.tile([128, T], FP, name="at2_ps", tag="big", bufs=2)
        for k in range(DC):
            nc.tensor.matmul(out=at2_ps,
                             lhsT=w["Wo_r"][:, k, dc * 128:(dc + 1) * 128],
                             rhs=aT[:, k, :], start=(k == 0), stop=(k == DC - 1))
        nc.vector.scalar_tensor_tensor(out=s1T[:, dc, :],
                                       in0=at2_ps,
                                       scalar=w["c_recv"][:, dc:dc + 1],
                                       in1=xTr[:, dc, :].bitcast(FP),
                                       op0=OP.add, op1=OP.add)
    out1T = pa.tile([128, DC, T], FPR, name="out1T", tag="big_b")
    layernorm_T(nc, w, ones_r, eps_t, pmi, pb3, ps,
                s1T, out1T, "ln1_g", "ln1_b", FPR, f"_l1_{gi}")

    # ---- MLP ----
    h2_ps = [ps.tile([128, T], FP, name=f"h2_ps{dc}", tag=f"h2_{dc}", bufs=1)
             for dc in range(DC)]
    for oc in range(OC):
        h1_ps = ps.tile([128, T], FP, name="h1_ps", tag="big", bufs=2)
        for k in range(DC):
            nc.tensor.matmul(out=h1_ps,
                             lhsT=w["W1"][:, k, oc * 128:(oc + 1) * 128],
                             rhs=out1T[:, k, :], start=(k == 0), stop=(k == DC - 1))
        gl = pb3.tile([128, T], FPR, name="gl", tag="gl")
        nc.scalar.activation(out=gl, in_=h1_ps, func=ACTF.Gelu,
                             bias=w["b1"][:, oc:oc + 1])
        for dc in range(DC):
            nc.tensor.matmul(out=h2_ps[dc],
                             lhsT=w["W2"][:, oc, dc * 128:(dc + 1) * 128],
                             rhs=gl, start=(oc == 0), stop=(oc == OC - 1))

    # ---- residual2 + LN2 -> outT (fp32, for output transposes) ----
    s2T = pa.tile([128, DC, T], FPR, name="s2T", tag="big_a")
    for dc in range(DC):
        nc.vector.scalar_tensor_tensor(out=s2T[:, dc, :],
                                       in0=h2_ps[dc],
                                       scalar=w["b2"][:, dc:dc + 1],
                                       in1=out1T[:, dc, :].bitcast(FP),
                                       op0=OP.add, op1=OP.add)
    outT = pa.tile([128, DC, T], FPR, name="outT", tag="outT")
    layernorm_T(nc, w, ones_r, eps_t, pmi, pb3, ps,
                s2T, outT, "ln2_g", "ln2_b", FP, f"_l2_{gi}")

    # ---- transpose back to token rows and store ----
    out_tok = po.tile([128, T // 128, D], FP, name="out_tok")
    for a in range(T // 128):
        for dc in range(DC):
            op_ps = ps.tile([128, 128], FPR, name="op_ps", tag="sp", bufs=2)
            nc.tensor.transpose(out=op_ps,
                                in_=outT[:, dc, a * 128:(a + 1) * 128],
                                identity=identr)
            nc.scalar.copy(out=out_tok[:, a, dc * 128:(dc + 1) * 128],
                           in_=op_ps.bitcast(FP))
    nc.gpsimd.dma_start(out=out[r0:r0 + T, :].rearrange("(a p) d -> p a d", p=128),
                        in_=out_tok)


def _host_fold(inputs):
    """Host-side weight-only precomputation."""
    f32 = np.float32
    scale = 1.0 / np.sqrt(np.float32(E))
    q_s = (inputs["router"] @ inputs["Wq_s"] + inputs["bq_s"]).astype(f32)
    q_sh = q_s.reshape(R, H, E)
    Wk = inputs["Wk_s"].reshape(D, H, E)
    M_score = (np.einsum("dhe,rhe->dhr", Wk, q_sh).reshape(D, HR) * scale).astype(f32)
    c_score = (np.einsum("he,rhe->hr", inputs["bk_s"].reshape(H, E), q_sh)
               .reshape(HR) * scale).astype(f32)
    c_send = (inputs["bv_s"] @ inputs["Wo_s"] + inputs["bo_s"]).astype(f32)
    c_recv = (inputs["bv_r"] @ inputs["Wo_r"] + inputs["bo_r"]).astype(f32)
    return {
        "Msc": np.ascontiguousarray(M_score),
        "c_score": c_score,
        "c_send": c_send,
        "c_recv": c_recv,
        "Wv_s": np.ascontiguousarray(inputs["Wv_s"].astype(f32)),
        "Wo_s": np.ascontiguousarray(inputs["Wo_s"].astype(f32)),
        "Wq_r": np.ascontiguousarray((inputs["Wq_r"] * scale).astype(f32)),
        "bq_r": (inputs["bq_r"] * scale).astype(f32),
        "Wk_r": np.ascontiguousarray(inputs["Wk_r"].astype(f32)),
        "bk_r": inputs["bk_r"].astype(f32),
        "Wv_r": np.ascontiguousarray(inputs["Wv_r"].astype(f32)),
        "Wo_r": np.ascontiguousarray(inputs["Wo_r"].astype(f32)),
        "W1": np.ascontiguousarray(inputs["W1"].astype(f32)),
        "b1": inputs["b1"].astype(f32),
        "W2": np.ascontiguousarray(inputs["W2"].astype(f32)),
        "b2": inputs["b2"].astype(f32),
        "ln1_g": inputs["ln1_g"].astype(f32),
        "ln1_b": inputs["ln1_b"].astype(f32),
        "ln2_g": inputs["ln2_g"].astype(f32),
        "ln2_b": inputs["ln2_b"].astype(f32),
    }


def kernel(**inputs) -> np.ndarray:
    inputs = {k: np.asarray(v) for k, v in inputs.items()}
    Z = inputs["Z"].astype(np.float32)
    n_seq_total = B * C
    n_seq = n_seq_total // N_CORES
    folded = _host_fold(inputs)

    nc = build_core_kernel(n_seq)
    Zf = Z.reshape(n_seq_total, L, D)
    in_maps = []
    for c in range(N_CORES):
        m = {"z": np.ascontiguousarray(
            Zf[c * n_seq:(c + 1) * n_seq].reshape(n_seq * L, D))}
        m.update(folded)
        in_maps.append(m)
    res = run_bass_kernel_spmd(nc, in_maps, list(range(N_CORES)))
    out = np.empty((n_seq_total, L, D), np.float32)
    for c in range(N_CORES):
        out[c * n_seq:(c + 1) * n_seq] = res.results[c]["out"].reshape(n_seq, L, D)
    return out.reshape(B, C, L, D)


if __name__ == "__main__":
    import reference
    inputs = reference.setup_inputs()
    inputs = {k: np.asarray(v) for k, v in inputs.items()}
    expected = np.asarray(reference.reference(**inputs))
    got = kernel(**inputs)
    err = np.abs(got - expected).max()
    rel = err / np.abs(expected).max()
    print(f"abs err {err:.3e}  absmax-rel {rel:.3e}")



# revision 10
# speedup vs baseline: 2.3113x; 2.3113x over previous
"""CrossPhaseRoutingLayer Trainium2 kernel (v2, bf16 datapath).

Full inputs -> full output. Data-parallel over the fused B*C=512 sequence axis
across 8 NeuronCores (64 sequences each); groups of G=4 sequences (T=384 token
columns) per pipeline stage.

Key structure (all weight-only folds validated against the reference):
  - Sender attention q is input-independent: scores fold to one matrix Msc;
    value/output path runs mix-first (Tm = A1 @ x), then per-head Wv_s, Wo_s.
  - Receiver q-projection folds through the router keys: scores = x @ (Wq_r *
    scale @ k^T), i.e. project the 8 routers, not the 384 tokens.  The bq_r
    contribution is a rank-1 matmul; the cross-sequence score blocks are
    killed with a rank-4 additive -100 mask before exp.
  - Receiver attention is batched over the group: one [128=(h,g,r), T] score
    matrix, one exp, one ones-matmul for denominators, one matmul per head to
    mix values.
  - ln1_g folds into W1 (and the final residual's g-mul rides the out1
    transpose copies); ln1_b folds into b1 and the h2 bias seed; b2 is seeded
    into the h2 PSUM accumulator with a K=1 matmul.
  - h2 runs token-oriented (lhsT = gelu tiles) so LN2 is a free-dim layernorm
    and the output needs no final transposes.
Everything runs in bf16 on the PE (fp32 PSUM accumulation); LN statistics and
softmax denominators stay fp32.  Tolerance is 2e-2 absmax-relative; measured
~1e-3.
"""
import numpy as np
import ml_dtypes

import concourse.bacc as bacc
import concourse.bass as bass
import concourse.mybir as mybir
import concourse.tile as tile
from concourse.bass_utils import run_bass_kernel_spmd
from concourse.masks import make_identity

FP = mybir.dt.float32
BF = mybir.dt.bfloat16
AX = mybir.AxisListType
OP = mybir.AluOpType
ACTF = mybir.ActivationFunctionType

B, C, L, D = 16, 32, 96, 512
R, H = 8, 4
E = D // H            # 128
HR = H * R            # 32
DC = D // 128         # 4 D-chunks
OC = (4 * D) // 128   # 16 MLP hidden chunks
EPS = 1e-5
N_CORES = 8
G = 4                 # sequences per group
T = G * L             # 384 token columns per group
TA = T // 128         # 3 token chunks of 128


def build_core_kernel(n_seq: int):
    """Bass program for one core processing n_seq sequences."""
    assert n_seq % G == 0
    n_groups = n_seq // G
    nc = bacc.Bacc(None)

    z = nc.declare_dram_parameter("z", [n_seq * L, D], BF, isOutput=False)
    out = nc.declare_dram_parameter("out", [n_seq * L, D], FP, isOutput=True)
    wd = {}
    # bf16 matmul weights, host-prearranged to [128, cols] SBUF layout
    for name, cols in [("Msc", DC * HR), ("Wv_s", DC * D), ("Wo_s", DC * D),
                       ("Wk_r", DC * D), ("Wv_r", DC * D), ("Wo_r", DC * D),
                       ("WqrT", H * D), ("bqr_e", H), ("W1", DC * 4 * D),
                       ("W2", OC * D), ("U4", None), ("V4", None),
                       ("Msum", None)]:
        if name == "U4":
            wd[name] = nc.declare_dram_parameter(name, [G, 128], BF, isOutput=False)
        elif name == "V4":
            wd[name] = nc.declare_dram_parameter(name, [G, T], BF, isOutput=False)
        elif name == "Msum":
            wd[name] = nc.declare_dram_parameter(name, [64, 2], BF, isOutput=False)
        else:
            wd[name] = nc.declare_dram_parameter(name, [128, cols], BF, isOutput=False)
    wd["b2row"] = nc.declare_dram_parameter("b2row", [1, D], BF, isOutput=False)
    # fp32 vectors
    for name, shape in [("c_score", [HR, 1]), ("c_send", [128, DC]),
                        ("bk_r", [128, DC]), ("c_recv", [128, DC]),
                        ("b1", [128, OC]), ("g1row", [1, D]),
                        ("g2row", [1, D]), ("b2brow", [1, D])]:
        wd[name] = nc.declare_dram_parameter(name, shape, FP, isOutput=False)

    with tile.TileContext(nc) as tc:
        with tc.tile_pool(name="wpool", bufs=1) as wp, \
             tc.tile_pool(name="xin", bufs=2) as px, \
             tc.tile_pool(name="act", bufs=2) as pa, \
             tc.tile_pool(name="sm", bufs=2) as psm, \
             tc.tile_pool(name="wk", bufs=2) as pb, \
             tc.tile_pool(name="ps", bufs=1, space="PSUM") as ps:

            # ---------------- resident weights / constants -----------------
            w = {}
            shapes = {"Msc": [128, DC, HR], "Wv_s": [128, DC, D],
                      "Wo_s": [128, DC, D], "Wk_r": [128, DC, D],
                      "Wv_r": [128, DC, D], "Wo_r": [128, DC, D],
                      "WqrT": [128, H, D], "bqr_e": [128, H],
                      "W1": [128, DC, 4 * D], "W2": [128, OC, D],
                      "U4": [G, 128], "V4": [G, T], "Msum": [64, 2],
                      "b2row": [1, D]}
            for name, shp in shapes.items():
                w[name] = wp.tile(shp, BF, name=f"w_{name}")
                nc.sync.dma_start(out=w[name],
                                  in_=wd[name].rearrange("p x -> p x"))
            for name, shp in [("c_score", [HR, 1]), ("c_send", [128, DC]),
                              ("bk_r", [128, DC]), ("c_recv", [128, DC]),
                              ("b1", [128, OC]), ("g1row", [1, D]),
                              ("g2row", [1, D]), ("b2brow", [1, D])]:
                w[name] = wp.tile(shp, FP, name=f"w_{name}")
                nc.sync.dma_start(out=w[name],
                                  in_=wd[name].rearrange("p x -> p x"))

            ident = wp.tile([128, 128], FP, name="ident")
            make_identity(nc, ident)
            identb = wp.tile([128, 128], BF, name="identb")
            nc.scalar.copy(out=identb, in_=ident)
            ones_f = wp.tile([128, 1], FP, name="ones_f")
            nc.vector.memset(ones_f, 1.0)
            onesb = wp.tile([128, 1], BF, name="onesb")
            nc.scalar.copy(out=onesb, in_=ones_f)
            onescol_b = wp.tile([1, 128], BF, name="onescol_b")
            nc.vector.memset(onescol_b, 1.0)
            ones_rowb = wp.tile([1, T], BF, name="ones_rowb")
            nc.vector.memset(ones_rowb, 1.0)
            eps_t = wp.tile([1, 1], FP, name="eps_t")
            nc.vector.memset(eps_t, EPS)
            eps_col = wp.tile([128, 1], FP, name="eps_col")
            nc.vector.memset(eps_col, EPS)

            # expanded per-(partition,chunk) biases -> [128, 128] tiles
            zt = wp.tile([128, HR], FP, name="zt")
            nc.vector.memset(zt, 0.0)
            w["c_sendX"] = wp.tile([128, DC, HR], FP, name="w_c_sendX")
            w["bk_rX"] = wp.tile([128, DC, HR], FP, name="w_bk_rX")
            for dc in range(DC):
                nc.vector.tensor_scalar_add(out=w["c_sendX"][:, dc, :],
                                            in0=zt,
                                            scalar1=w["c_send"][:, dc:dc + 1])
                nc.vector.tensor_scalar_add(out=w["bk_rX"][:, dc, :],
                                            in0=zt,
                                            scalar1=w["bk_r"][:, dc:dc + 1])
            # token-layout row broadcasts
            for name, src in [("g1B", "g1row"), ("g2B", "g2row"),
                              ("b2bB", "b2brow")]:
                w[name] = wp.tile([128, D], FP, name=f"w_{name}")
                nc.gpsimd.partition_broadcast(w[name], w[src])

            for gi in range(n_groups):
                group_body(nc, tc, w, ident, identb, onesb, onescol_b,
                           ones_rowb, eps_t, eps_col, z, out, gi,
                           px, pa, psm, pb, ps)
    nc.finalize()
    return nc


def group_body(nc, tc, w, ident, identb, onesb, onescol_b, ones_rowb,
               eps_t, eps_col, z, out, gi, px, pa, psm, pb, ps):
    r0 = gi * T   # first DRAM row of the group

    # ---- load x (token-partition) and build x^T ----
    x_tok = px.tile([L, G, D], BF, name="x_tok")
    nc.sync.dma_start(out=x_tok,
                      in_=z[r0:r0 + T, :].rearrange("(g l) d -> l g d", g=G))
    xT = pa.tile([128, DC, T], BF, name="xT")
    for dc in range(DC):
        pt = ps.tile([128, G, L], BF, name="pt_x", tag="tp", bufs=2)
        for g in range(G):
            nc.tensor.transpose(out=pt[:, g, :],
                                in_=x_tok[:, g, dc * 128:(dc + 1) * 128],
                                identity=identb[:L, :L])
        nc.scalar.copy(out=xT[:, dc, :], in_=pt.rearrange("p g l -> p (g l)"))

    # ---- sender scores^T [HR, T], softmax over tokens, A1^T ----
    sc_ps = ps.tile([HR, T], FP, name="sc_ps", tag="big", bufs=3)
    for k in range(DC):
        nc.tensor.matmul(out=sc_ps, lhsT=w["Msc"][:, k, :], rhs=xT[:, k, :],
                         start=(k == 0), stop=(k == DC - 1))
    e1 = psm.tile([HR, T], BF, name="e1")
    nc.scalar.activation(out=e1, in_=sc_ps, func=ACTF.Exp, bias=w["c_score"])
    s1sum = psm.tile([HR, G], FP, name="s1sum")
    nc.vector.tensor_reduce(out=s1sum, in_=e1.rearrange("p (g l) -> p g l", g=G),
                            axis=AX.X, op=OP.add)
    r1 = psm.tile([HR, G], FP, name="r1")
    nc.vector.reciprocal_approx_fast(out=r1, in_=s1sum)
    a1p = ps.tile([L, G, HR], BF, name="a1p", tag="tp", bufs=2)
    for g in range(G):
        a1n = psm.tile([HR, L], BF, name=f"a1n{g}", tag="a1n", bufs=2)
        nc.vector.tensor_scalar_mul(out=a1n, in0=e1[:, g * L:(g + 1) * L],
                                    scalar1=r1[:, g:g + 1])
        nc.tensor.transpose(out=a1p[:, g, :], in_=a1n, identity=identb[:HR, :HR])
    a1s = psm.tile([L, G, HR], BF, name="a1s")
    nc.scalar.copy(out=a1s, in_=a1p)

    # ---- T_mix^T [128, (dc, g, hr)] = x_chunk.T @ A1^T ----
    tm_ps = ps.tile([128, DC, G, HR], FP, name="tm_ps", tag="big", bufs=3)
    for dc in range(DC):
        for g in range(G):
            nc.tensor.matmul(out=tm_ps[:, dc, g, :],
                             lhsT=x_tok[:, g, dc * 128:(dc + 1) * 128],
                             rhs=a1s[:, g, :], start=True, stop=True)
    TmT = psm.tile([128, DC, G, HR], BF, name="TmT")
    nc.scalar.copy(out=TmT, in_=tm_ps)

    # ---- out_cat^T chunk h = Wv_s_h^T @ Tm_h^T   [128, (h, g, r)] ----
    oc_ps = ps.tile([128, H, G, R], FP, name="oc_ps", tag="big", bufs=3)
    for h in range(H):
        for k in range(DC):
            nc.tensor.matmul(out=oc_ps[:, h, :, :],
                             lhsT=w["Wv_s"][:, k, h * E:(h + 1) * E],
                             rhs=TmT[:, k, :, h * R:(h + 1) * R],
                             start=(k == 0), stop=(k == DC - 1))
    Oc = psm.tile([128, H, G, R], BF, name="Oc")
    nc.scalar.copy(out=Oc, in_=oc_ps)

    # ---- router_buffer^T [128, (dc, g, r)] = Wo_s^T @ out_cat^T + c_send ----
    rb_ps = ps.tile([128, DC, G, R], FP, name="rb_ps", tag="big", bufs=3)
    for dc in range(DC):
        for k in range(DC):
            nc.tensor.matmul(out=rb_ps[:, dc, :, :],
                             lhsT=w["Wo_s"][:, k, dc * 128:(dc + 1) * 128],
                             rhs=Oc[:, k, :, :],
                             start=(k == 0), stop=(k == DC - 1))
    # rb4: bias-added router buffer, replicated 4x along an h axis so the
    # batched v matmul emits v at every 32-partition offset (mix lhsT/rhs
    # must share a base partition).
    rb4 = psm.tile([128, DC, H, G, R], BF, name="rb4")
    for h in range(H):
        nc.vector.tensor_add(out=rb4[:, :, h, :, :], in0=rb_ps,
                             in1=w["c_sendX"].rearrange("p c x -> p (c x)")
                             .rearrange("p (c g r) -> p c g r", c=DC, g=G))

    # ---- receiver k^T [128, (h, g, r)] (chunk dc == head h since E=128) ----
    kt_ps = ps.tile([128, DC, G, R], FP, name="kt_ps", tag="big", bufs=3)
    for dc in range(DC):
        for k in range(DC):
            nc.tensor.matmul(out=kt_ps[:, dc, :, :],
                             lhsT=w["Wk_r"][:, k, dc * 128:(dc + 1) * 128],
                             rhs=rb4[:, k, 0, :, :],
                             start=(k == 0), stop=(k == DC - 1))
    kT = psm.tile([128, DC, G, R], BF, name="kT")
    nc.vector.tensor_add(out=kT, in0=kt_ps,
                         in1=w["bk_rX"].rearrange("p c x -> p (c x)")
                         .rearrange("p (c g r) -> p c g r", c=DC, g=G))

    # ---- fold Wq_r through k:  Wtil[d, (h,g,r)] = Wq_r_h @ k_h^T ----
    wt_ps = ps.tile([128, DC, H, G * R], FP, name="wt_ps", tag="big", bufs=3)
    for dc in range(DC):
        for h in range(H):
            nc.tensor.matmul(out=wt_ps[:, dc, h, :],
                             lhsT=w["WqrT"][:, h, dc * 128:(dc + 1) * 128],
                             rhs=kT[:, h, :, :], start=True, stop=True)
    Wtil = psm.tile([128, DC, H * G * R], BF, name="Wtil")
    nc.scalar.copy(out=Wtil, in_=wt_ps.rearrange("p c h x -> p c (h x)"))

    # ---- bq_r . k  per (h,g,r): rank-1 score bias row ----
    br_ps = ps.tile([1, H, G * R], FP, name="br_ps", tag="tp", bufs=2)
    for h in range(H):
        nc.tensor.matmul(out=br_ps[:, h, :],
                         lhsT=w["bqr_e"][:, h:h + 1],
                         rhs=kT[:, h, :, :], start=True, stop=True)
    brow = psm.tile([1, H * G * R], BF, name="brow")
    nc.scalar.copy(out=brow, in_=br_ps.rearrange("p h x -> p (h x)"))

    # ---- receiver scores [128=(h,g,r), T]: x-proj + bias + block mask ----
    s2_ps = ps.tile([128, T], FP, name="s2_ps", tag="big", bufs=3)
    for k in range(DC):
        nc.tensor.matmul(out=s2_ps, lhsT=Wtil[:, k, :], rhs=xT[:, k, :],
                         start=(k == 0), stop=False)
    nc.tensor.matmul(out=s2_ps, lhsT=brow, rhs=ones_rowb,
                     start=False, stop=False)
    nc.tensor.matmul(out=s2_ps, lhsT=w["U4"], rhs=w["V4"],
                     start=False, stop=True)
    e2a = psm.tile([64, T], BF, name="e2a")
    nc.scalar.activation(out=e2a, in_=s2_ps[0:64, :], func=ACTF.Exp)
    e2b = psm.tile([64, T], BF, name="e2b")
    nc.scalar.activation(out=e2b, in_=s2_ps[64:128, :], func=ACTF.Exp)

    # ---- denominators per (h, token) + reciprocal ----
    r2h = []
    for h in range(H):
        base = (h % 2) * HR
        den_h = ps.tile([1, T], FP, name=f"den{h}", tag="tp", bufs=2)
        nc.tensor.matmul(out=den_h, lhsT=onesb[base:base + HR, :],
                         rhs=[e2a, e2b][h // 2][base:base + HR, :],
                         start=True, stop=True)
        rh = psm.tile([1, T], FP, name=f"r2_{h}", tag="r2h", bufs=4)
        nc.vector.reciprocal_approx_fast(out=rh, in_=den_h)
        r2h.append(rh)

    # ---- receiver v [32=(g,r), D] (batched over the group) ----
    v_ps = ps.tile([128, D], FP, name="v_ps", tag="big", bufs=3)
    for k in range(DC):
        nc.tensor.matmul(out=v_ps,
                         lhsT=rb4[:, k, :, :, :].rearrange("p h g r -> p (h g r)"),
                         rhs=w["Wv_r"][:, k, :],
                         start=(k == 0), stop=(k == DC - 1))
    v_sb = psm.tile([128, D], BF, name="v_sb")
    nc.scalar.copy(out=v_sb, in_=v_ps)

    # ---- mix + normalize -> attn_pre^T [128, (h), T] ----
    aT = pa.tile([128, H, T], BF, name="aT")
    e2ab = [e2a, e2b]
    for h in range(H):
        recB = pb.tile([128, T], FP, name=f"recB{h}", tag="recB", bufs=2)
        nc.gpsimd.partition_broadcast(recB, r2h[h])
        base = (h % 2) * HR
        mx_ps = ps.tile([128, T], FP, name="mx_ps", tag="big", bufs=3)
        nc.tensor.matmul(out=mx_ps,
                         lhsT=v_sb[base:base + HR, h * E:(h + 1) * E],
                         rhs=e2ab[h // 2][base:base + HR, :],
                         start=True, stop=True)
        nc.vector.tensor_mul(out=aT[:, h, :], in0=mx_ps, in1=recB)

    # ---- attn2^T = Wo_r^T @ attn_pre^T + c_recv; residual -> s1T ----
    s1T = pa.tile([128, DC, T], BF, name="s1T")
    for dc in range(DC):
        at_ps = ps.tile([128, T], FP, name="at_ps", tag="big", bufs=3)
        for k in range(DC):
            nc.tensor.matmul(out=at_ps,
                             lhsT=w["Wo_r"][:, k, dc * 128:(dc + 1) * 128],
                             rhs=aT[:, k, :], start=(k == 0), stop=(k == DC - 1))
        nc.vector.scalar_tensor_tensor(out=s1T[:, dc, :],
                                       in0=at_ps,
                                       scalar=w["c_recv"][:, dc:dc + 1],
                                       in1=xT[:, dc, :],
                                       op0=OP.add, op1=OP.add)

    # ---- LN1 (partition-split D axis), gains folded into W1/W2-seed ----
    mean_ps = ps.tile([1, T], FP, name="mean_ps", tag="tp", bufs=2)
    for k in range(DC):
        nc.tensor.matmul(out=mean_ps, lhsT=onesb, rhs=s1T[:, k, :],
                         start=(k == 0), stop=(k == DC - 1))
    msc = psm.tile([1, T], FP, name="msc")
    nc.scalar.activation(out=msc, in_=mean_ps, func=ACTF.Copy, scale=1.0 / D)
    sqt = pb.tile([128, DC, T], BF, name="sqt", tag="sqt", bufs=2)
    nc.vector.tensor_mul(out=sqt.rearrange("p c t -> p (c t)"),
                         in0=s1T.rearrange("p c t -> p (c t)"),
                         in1=s1T.rearrange("p c t -> p (c t)"))
    ss_ps = ps.tile([1, T], FP, name="ss_ps", tag="tp", bufs=2)
    for k in range(DC):
        nc.tensor.matmul(out=ss_ps, lhsT=onesb, rhs=sqt[:, k, :],
                         start=(k == 0), stop=(k == DC - 1))
    msc2 = psm.tile([1, T], FP, name="msc2")
    nc.vector.tensor_mul(out=msc2, in0=msc, in1=msc)
    var_s = psm.tile([1, T], FP, name="var_s")
    nc.vector.scalar_tensor_tensor(out=var_s, in0=ss_ps, scalar=1.0 / D,
                                   in1=msc2, op0=OP.mult, op1=OP.subtract)
    srt = psm.tile([1, T], FP, name="srt")
    nc.scalar.activation(out=srt, in_=var_s, func=ACTF.Sqrt, bias=eps_t)
    rstd = psm.tile([1, T], FP, name="rstd")
    nc.vector.reciprocal_approx_fast(out=rstd, in_=srt)
    rstdB = pb.tile([128, T], FP, name="rstdB", tag="rstdB", bufs=2)
    nc.gpsimd.partition_broadcast(rstdB, rstd)
    mscB = pb.tile([128, T], FP, name="mscB", tag="mscB", bufs=2)
    nc.gpsimd.partition_broadcast(mscB, msc)
    out1T = pa.tile([128, DC, T], BF, name="out1T")
    for dc in range(DC):
        t1 = pb.tile([128, T], FP, name="t1", tag="t1", bufs=2)
        nc.vector.tensor_sub(out=t1, in0=s1T[:, dc, :], in1=mscB)
        nc.vector.tensor_mul(out=out1T[:, dc, :], in0=t1, in1=rstdB)

    # ---- transpose out1 to token rows (g1-scaled) for the residual ----
    out1_tok = pa.tile([128, TA, D], BF, name="out1_tok")
    for a in range(TA):
        tr_ps = ps.tile([128, D], BF, name="tr_ps", tag="tp", bufs=2)
        for dc in range(DC):
            nc.tensor.transpose(out=tr_ps[:, dc * 128:(dc + 1) * 128],
                                in_=out1T[:, dc, a * 128:(a + 1) * 128],
                                identity=identb)
        nc.vector.tensor_mul(out=out1_tok[:, a, :], in0=tr_ps, in1=w["g1B"])

    # ---- MLP: h1 (d-part) -> gelu -> h2 (token-oriented, b2 seeded) ----
    h2_ps = [ps.tile([128, D], FP, name=f"h2_ps{a}", tag=f"h2_{a}", bufs=1)
             for a in range(TA)]
    for a in range(TA):
        nc.tensor.matmul(out=h2_ps[a], lhsT=onescol_b, rhs=w["b2row"],
                         start=True, stop=False)
    for oc in range(OC):
        h1_ps = ps.tile([128, T], FP, name="h1_ps", tag="big", bufs=3)
        for k in range(DC):
            nc.tensor.matmul(out=h1_ps,
                             lhsT=w["W1"][:, k, oc * 128:(oc + 1) * 128],
                             rhs=out1T[:, k, :], start=(k == 0), stop=(k == DC - 1))
        gl = pb.tile([128, T], BF, name="gl", tag="gl", bufs=3)
        nc.scalar.activation(out=gl, in_=h1_ps, func=ACTF.Gelu,
                             bias=w["b1"][:, oc:oc + 1])
        for a in range(TA):
            nc.tensor.matmul(out=h2_ps[a],
                             lhsT=gl[:, a * 128:(a + 1) * 128],
                             rhs=w["W2"][:, oc, :],
                             start=False, stop=(oc == OC - 1))

    # ---- residual2 + LN2 (token layout, free-dim reduce) ----
    s2t = pb.tile([128, TA, D], FP, name="s2t", tag="s2t", bufs=2)
    for a in range(TA):
        nc.vector.tensor_add(out=s2t[:, a, :], in0=h2_ps[a],
                             in1=out1_tok[:, a, :])
    sum2 = psm.tile([128, TA], FP, name="sum2")
    nc.vector.tensor_reduce(out=sum2, in_=s2t, axis=AX.X, op=OP.add)
    m2t = psm.tile([128, TA], FP, name="m2t")
    nc.vector.tensor_scalar_mul(out=m2t, in0=sum2, scalar1=1.0 / D)
    sq2 = pb.tile([128, TA, D], BF, name="sq2", tag="sq2", bufs=2)
    nc.vector.tensor_mul(out=sq2.rearrange("p a d -> p (a d)"),
                         in0=s2t.rearrange("p a d -> p (a d)"),
                         in1=s2t.rearrange("p a d -> p (a d)"))
    ssum2 = psm.tile([128, TA], FP, name="ssum2")
    nc.vector.tensor_reduce(out=ssum2, in_=sq2, axis=AX.X, op=OP.add)
    mm2 = psm.tile([128, TA], FP, name="mm2")
    nc.vector.tensor_mul(out=mm2, in0=m2t, in1=m2t)
    var2 = psm.tile([128, TA], FP, name="var2")
    nc.vector.scalar_tensor_tensor(out=var2, in0=ssum2, scalar=1.0 / D,
                                   in1=mm2, op0=OP.mult, op1=OP.subtract)
    srt2 = psm.tile([128, TA], FP, name="srt2")
    nc.scalar.activation(out=srt2, in_=var2, func=ACTF.Sqrt, bias=eps_col)
    rstd2 = psm.tile([128, TA], FP, name="rstd2")
    nc.vector.reciprocal_approx_fast(out=rstd2, in_=srt2)
    out_tok = pa.tile([128, TA, D], FP, name="out_tok")
    for a in range(TA):
        xc = pb.tile([128, D], FP, name="xc", tag="xc", bufs=2)
        nc.vector.tensor_scalar(out=xc, in0=s2t[:, a, :],
                                scalar1=m2t[:, a:a + 1], op0=OP.subtract,
                                scalar2=rstd2[:, a:a + 1], op1=OP.mult)
        gx = pb.tile([128, D], FP, name="gx", tag="gx", bufs=2)
        nc.gpsimd.tensor_mul(out=gx, in0=xc, in1=w["g2B"])
        nc.gpsimd.tensor_add(out=out_tok[:, a, :], in0=gx, in1=w["b2bB"])
    nc.gpsimd.dma_start(out=out[r0:r0 + T, :].rearrange("(a p) d -> p a d", p=128),
                        in_=out_tok)


def _host_fold(inputs):
    """Host-side weight-only precomputation (bf16 for matmul operands)."""
    f32 = np.float32
    bf = ml_dtypes.bfloat16
    scale = 1.0 / np.sqrt(np.float32(E))

    def chunked(a):
        # [D_in, X] -> [128, DC_in * X] partition-major chunk layout
        d_in, x = a.shape
        c = d_in // 128
        return np.ascontiguousarray(
            a.reshape(c, 128, x).transpose(1, 0, 2).reshape(128, c * x))

    q_s = (inputs["router"] @ inputs["Wq_s"] + inputs["bq_s"]).astype(f32)
    q_sh = q_s.reshape(R, H, E)
    Wk = inputs["Wk_s"].reshape(D, H, E)
    M_score = (np.einsum("dhe,rhe->dhr", Wk, q_sh).reshape(D, HR) * scale).astype(f32)
    c_score = (np.einsum("he,rhe->hr", inputs["bk_s"].reshape(H, E), q_sh)
               .reshape(HR) * scale).astype(f32)
    c_send = (inputs["bv_s"] @ inputs["Wo_s"] + inputs["bo_s"]).astype(f32)
    c_recv = (inputs["bv_r"] @ inputs["Wo_r"] + inputs["bo_r"]).astype(f32)

    WqrT = (inputs["Wq_r"].astype(f32) * scale).reshape(D, H, E).transpose(2, 1, 0)
    WqrT = np.ascontiguousarray(WqrT.reshape(128, H * D))
    bqr_e = np.ascontiguousarray(
        (inputs["bq_r"].astype(f32) * scale).reshape(H, E).T)

    W1p = inputs["ln1_g"][:, None].astype(f32) * inputs["W1"].astype(f32)
    b1p = (inputs["b1"].astype(f32)
           + inputs["ln1_b"].astype(f32) @ inputs["W1"].astype(f32))
    b2row = (inputs["b2"].astype(f32) + inputs["ln1_b"].astype(f32))

    U4 = np.zeros((G, 128), f32)
    V4 = np.zeros((G, T), f32)
    for g in range(G):
        for p in range(128):
            if (p % HR) // R == g:
                U4[g, p] = 1.0
        V4[g, :] = -100.0
        V4[g, g * L:(g + 1) * L] = 0.0
    Msum = np.zeros((64, 2), f32)
    for p in range(64):
        Msum[p, p // HR] = 1.0

    def colvec(v):  # [D] -> [128, DC] (partition p, chunk c) = v[c*128+p]
        return np.ascontiguousarray(v.reshape(DC, 128).T.astype(f32))

    return {
        "Msc": chunked(M_score).astype(bf),
        "c_score": c_score.reshape(HR, 1),
        "c_send": colvec(c_send),
        "c_recv": colvec(c_recv),
        "bk_r": colvec(inputs["bk_r"].astype(f32)),
        "Wv_s": chunked(inputs["Wv_s"].astype(f32)).astype(bf),
        "Wo_s": chunked(inputs["Wo_s"].astype(f32)).astype(bf),
        "Wk_r": chunked(inputs["Wk_r"].astype(f32)).astype(bf),
        "Wv_r": chunked(inputs["Wv_r"].astype(f32)).astype(bf),
        "Wo_r": chunked(inputs["Wo_r"].astype(f32)).astype(bf),
        "WqrT": WqrT.astype(bf),
        "bqr_e": bqr_e.astype(bf),
        "W1": chunked(W1p).astype(bf),
        "b1": np.ascontiguousarray(b1p.reshape(OC, 128).T),
        "W2": chunked(inputs["W2"].astype(f32)).astype(bf),
        "b2row": b2row.reshape(1, D).astype(bf),
        "U4": U4.astype(bf),
        "V4": V4.astype(bf),
        "Msum": Msum.astype(bf),
        "g1row": inputs["ln1_g"].astype(f32).reshape(1, D),
        "g2row": inputs["ln2_g"].astype(f32).reshape(1, D),
        "b2brow": inputs["ln2_b"].astype(f32).reshape(1, D),
    }


def _core_in_maps(Z, folded):
    """Per-core input maps (Z full fp32 array [B, C, L, D])."""
    n_seq_total = B * C
    n_seq = n_seq_total // N_CORES
    Zb = Z.reshape(n_seq_total, L, D).astype(ml_dtypes.bfloat16)
    in_maps = []
    for c in range(N_CORES):
        m = {"z": np.ascontiguousarray(
            Zb[c * n_seq:(c + 1) * n_seq].reshape(n_seq * L, D))}
        m.update(folded)
        in_maps.append(m)
    return in_maps


def kernel(**inputs) -> np.ndarray:
    inputs = {k: np.asarray(v) for k, v in inputs.items()}
    Z = inputs["Z"].astype(np.float32)
    n_seq_total = B * C
    n_seq = n_seq_total // N_CORES
    folded = _host_fold(inputs)

    nc = build_core_kernel(n_seq)
    in_maps = _core_in_maps(Z, folded)
    res = run_bass_kernel_spmd(nc, in_maps, list(range(N_CORES)))
    out = np.empty((n_seq_total, L, D), np.float32)
    for c in range(N_CORES):
        out[c * n_seq:(c + 1) * n_seq] = res.results[c]["out"].reshape(n_seq, L, D)
    return out.reshape(B, C, L, D)


if __name__ == "__main__":
    import reference
    inputs = reference.setup_inputs()
    inputs = {k: np.asarray(v) for k, v in inputs.items()}
    expected = np.asarray(reference.reference(**inputs))
    got = kernel(**inputs)
    err = np.abs(got - expected).max()
    rel = err / np.abs(expected).max()
    print(f"abs err {err:.3e}  absmax-rel {rel:.3e}")


# revision 12
# speedup vs baseline: 2.3431x; 1.0138x over previous
"""CrossPhaseRoutingLayer Trainium2 kernel (v3, bf16 + software pipelining).

Full inputs -> full output. Data-parallel over the fused B*C=512 sequence axis
across 8 NeuronCores (64 sequences each); groups of G=4 sequences (T=384 token
columns) per pipeline stage.

Structure (all weight-only folds validated against the reference):
  - Sender attention q is input-independent: scores fold to one matrix Msc;
    value/output path runs mix-first (Tm = A1 @ x), then per-head Wv_s, Wo_s.
  - Receiver q-projection folds through the router keys: scores = x @ (Wq_r *
    scale @ k^T) — project the 8 routers, not the 384 tokens.  bq_r enters as
    a rank-1 matmul; cross-sequence score blocks are killed with a rank-4
    additive -100 mask before exp.
  - Receiver attention batched over the group: [128=(h,g,r), T] scores, one
    exp per 64-partition half, ones-matmul denominators, one mix matmul/head.
  - ln1_g folds into W1 and into a per-chunk diagonal matrix Idg1 used to
    project out1 back to token rows INTO the h2 PSUM accumulator (residual
    add for free); ln1_b folds into b1 and the h2 bias seed; b2 is seeded
    into h2 PSUM with a K=1 matmul.  LN2 is a free-dim layernorm reading the
    h2 PSUM directly; the output needs no final transposes.
  - Emission is software-pipelined: group i's MLP/LN2 instructions interleave
    with group i+1's front-end so the in-order engines never stall long
    enough to re-throttle the PE clock (HAM).
Everything runs bf16 on the PE (fp32 PSUM accumulation); LN statistics and
softmax denominators stay fp32.
"""
import numpy as np
import ml_dtypes

import concourse.bacc as bacc
import concourse.bass as bass
import concourse.mybir as mybir
import concourse.tile as tile
from concourse.bass_utils import run_bass_kernel_spmd
from concourse.masks import make_identity

FP = mybir.dt.float32
BF = mybir.dt.bfloat16
AX = mybir.AxisListType
OP = mybir.AluOpType
ACTF = mybir.ActivationFunctionType

B, C, L, D = 16, 32, 96, 512
R, H = 8, 4
E = D // H            # 128
HR = H * R            # 32
DC = D // 128         # 4 D-chunks
OC = (4 * D) // 128   # 16 MLP hidden chunks
EPS = 1e-5
N_CORES = 8
G = 4                 # sequences per group
T = G * L             # 384 token columns per group
TA = T // 128         # 3 token chunks of 128

BF_NAMES = {"Msc": [128, DC * HR], "Wv_s": [128, DC * D], "Wo_s": [128, DC * D],
            "Wk_r": [128, DC * D], "Wv_r": [128, DC * D], "Wo_r": [128, DC * D],
            "WqrT": [128, H * D], "bqr_e": [128, H], "W1": [128, DC * 4 * D],
            "W2": [128, OC * D], "U4": [G, 128], "V4": [G, T],
            "Msum": [64, 2], "b2row": [1, D], "Idg1": [128, DC * 128]}
FP_NAMES = {"c_score": [HR, 1], "c_send": [128, DC], "bk_r": [128, DC],
            "c_recv": [128, DC], "b1": [128, OC], "g2row": [1, D],
            "b2brow": [1, D]}
SB_SHAPES = {"Msc": [128, DC, HR], "Wv_s": [128, DC, D], "Wo_s": [128, DC, D],
             "Wk_r": [128, DC, D], "Wv_r": [128, DC, D], "Wo_r": [128, DC, D],
             "WqrT": [128, H, D], "bqr_e": [128, H], "W1": [128, DC, 4 * D],
             "W2": [128, OC, D], "U4": [G, 128], "V4": [G, T],
             "Msum": [64, 2], "b2row": [1, D], "Idg1": [128, DC, 128]}


def build_core_kernel(n_seq: int):
    """Bass program for one core processing n_seq sequences."""
    assert n_seq % G == 0
    n_groups = n_seq // G
    nc = bacc.Bacc(None)

    z = nc.declare_dram_parameter("z", [n_seq * L, D], BF, isOutput=False)
    out = nc.declare_dram_parameter("out", [n_seq * L, D], FP, isOutput=True)
    wd = {}
    for name, shp in BF_NAMES.items():
        wd[name] = nc.declare_dram_parameter(name, shp, BF, isOutput=False)
    for name, shp in FP_NAMES.items():
        wd[name] = nc.declare_dram_parameter(name, shp, FP, isOutput=False)

    with tile.TileContext(nc) as tc:
        with tc.tile_pool(name="wpool", bufs=1) as wp, \
             tc.tile_pool(name="xin", bufs=3) as px, \
             tc.tile_pool(name="act", bufs=2) as pa, \
             tc.tile_pool(name="sm", bufs=2) as psm, \
             tc.tile_pool(name="wk", bufs=2) as pb, \
             tc.tile_pool(name="ps", bufs=1, space="PSUM") as ps:

            # ---------------- resident weights / constants -----------------
            w = {}
            for name, shp in SB_SHAPES.items():
                w[name] = wp.tile(shp, BF, name=f"w_{name}")
                nc.sync.dma_start(out=w[name],
                                  in_=wd[name].rearrange("p x -> p x"))
            for name, shp in FP_NAMES.items():
                w[name] = wp.tile(shp, FP, name=f"w_{name}")
                nc.sync.dma_start(out=w[name],
                                  in_=wd[name].rearrange("p x -> p x"))

            ident = wp.tile([128, 128], FP, name="ident")
            make_identity(nc, ident)
            identb = wp.tile([128, 128], BF, name="identb")
            nc.scalar.copy(out=identb, in_=ident)
            ones_f = wp.tile([128, 1], FP, name="ones_f")
            nc.vector.memset(ones_f, 1.0)
            onesb = wp.tile([128, 1], BF, name="onesb")
            nc.scalar.copy(out=onesb, in_=ones_f)
            onescol_b = wp.tile([1, 128], BF, name="onescol_b")
            nc.vector.memset(onescol_b, 1.0)
            ones_rowb = wp.tile([1, T], BF, name="ones_rowb")
            nc.vector.memset(ones_rowb, 1.0)
            eps_t = wp.tile([1, 1], FP, name="eps_t")
            nc.vector.memset(eps_t, EPS)
            eps_col = wp.tile([128, 1], FP, name="eps_col")
            nc.vector.memset(eps_col, EPS)

            zt = wp.tile([128, HR], FP, name="zt")
            nc.vector.memset(zt, 0.0)
            w["c_sendX"] = wp.tile([128, DC, HR], FP, name="w_c_sendX")
            w["bk_rX"] = wp.tile([128, DC, HR], FP, name="w_bk_rX")
            for dc in range(DC):
                nc.vector.tensor_scalar_add(out=w["c_sendX"][:, dc, :],
                                            in0=zt,
                                            scalar1=w["c_send"][:, dc:dc + 1])
                nc.vector.tensor_scalar_add(out=w["bk_rX"][:, dc, :],
                                            in0=zt,
                                            scalar1=w["bk_r"][:, dc:dc + 1])
            for name, src in [("g2B", "g2row"), ("b2bB", "b2brow")]:
                w[name] = wp.tile([128, D], FP, name=f"w_{name}")
                nc.gpsimd.partition_broadcast(w[name], w[src])

            cst = dict(identb=identb, onesb=onesb, onescol_b=onescol_b,
                       ones_rowb=ones_rowb, eps_t=eps_t, eps_col=eps_col)
            pools = dict(px=px, pa=pa, psm=psm, pb=pb, ps=ps)

            # software pipeline: interleave B(i-1) with A(i)
            prevB = None
            for gi in range(n_groups):
                st = {}
                A = gen_A(nc, w, cst, pools, z, gi, st)
                _interleave(prevB, A)
                prevB = gen_B(nc, w, cst, pools, out, gi, st)
            _interleave(prevB, None)
    nc.finalize()
    return nc


def _interleave(g1, g2):
    its = [it for it in (g1, g2) if it is not None]
    while its:
        nxt = []
        for it in its:
            try:
                next(it)
                nxt.append(it)
            except StopIteration:
                pass
        its = nxt


def gen_A(nc, w, cst, pools, z, gi, st):
    """Front-end: x load/transpose, sender attention, receiver attention,
    residual 1, LN1 -> out1T.  Yields between chunks for interleaving."""
    px, pa, psm, pb, ps = (pools[k] for k in ("px", "pa", "psm", "pb", "ps"))
    identb, onesb = cst["identb"], cst["onesb"]
    r0 = gi * T

    x_tok = px.tile([L, G, D], BF, name="x_tok")
    nc.sync.dma_start(out=x_tok,
                      in_=z[r0:r0 + T, :].rearrange("(g l) d -> l g d", g=G))
    yield

    xT = pa.tile([128, DC, T], BF, name="xT")
    st["xT"] = xT
    for dc0 in (0, 2):
        for dc in (dc0, dc0 + 1):
            pt = ps.tile([128, G, L], BF, name="pt_x", tag="tp", bufs=2)
            for g in range(G):
                nc.tensor.transpose(out=pt[:, g, :],
                                    in_=x_tok[:, g, dc * 128:(dc + 1) * 128],
                                    identity=identb[:L, :L])
            nc.scalar.copy(out=xT[:, dc, :],
                           in_=pt.rearrange("p g l -> p (g l)"))
        yield

    # sender scores + softmax
    sc_ps = ps.tile([HR, T], FP, name="sc_ps", tag="big", bufs=3)
    for k in range(DC):
        nc.tensor.matmul(out=sc_ps, lhsT=w["Msc"][:, k, :], rhs=xT[:, k, :],
                         start=(k == 0), stop=(k == DC - 1))
    e1 = psm.tile([HR, T], BF, name="e1")
    nc.scalar.activation(out=e1, in_=sc_ps, func=ACTF.Exp, bias=w["c_score"])
    yield

    s1sum = psm.tile([HR, G], FP, name="s1sum")
    nc.vector.tensor_reduce(out=s1sum, in_=e1.rearrange("p (g l) -> p g l", g=G),
                            axis=AX.X, op=OP.add)
    r1 = psm.tile([HR, G], FP, name="r1")
    nc.vector.reciprocal_approx_fast(out=r1, in_=s1sum)
    a1p = ps.tile([L, G, HR], BF, name="a1p", tag="tp", bufs=2)
    for g in range(G):
        a1n = psm.tile([HR, L], BF, name=f"a1n{g}", tag="a1n", bufs=2)
        nc.vector.tensor_scalar_mul(out=a1n, in0=e1[:, g * L:(g + 1) * L],
                                    scalar1=r1[:, g:g + 1])
        nc.tensor.transpose(out=a1p[:, g, :], in_=a1n, identity=identb[:HR, :HR])
    a1s = psm.tile([L, G, HR], BF, name="a1s")
    nc.scalar.copy(out=a1s, in_=a1p)
    yield

    # Tm
    tm_ps = ps.tile([128, DC, G, HR], FP, name="tm_ps", tag="big", bufs=3)
    for dc in range(DC):
        for g in range(G):
            nc.tensor.matmul(out=tm_ps[:, dc, g, :],
                             lhsT=x_tok[:, g, dc * 128:(dc + 1) * 128],
                             rhs=a1s[:, g, :], start=True, stop=True)
    TmT = psm.tile([128, DC, G, HR], BF, name="TmT")
    nc.scalar.copy(out=TmT, in_=tm_ps)
    yield

    # Oc
    oc_ps = ps.tile([128, H, G, R], FP, name="oc_ps", tag="big", bufs=3)
    for h in range(H):
        for k in range(DC):
            nc.tensor.matmul(out=oc_ps[:, h, :, :],
                             lhsT=w["Wv_s"][:, k, h * E:(h + 1) * E],
                             rhs=TmT[:, k, :, h * R:(h + 1) * R],
                             start=(k == 0), stop=(k == DC - 1))
    Oc = psm.tile([128, H, G, R], BF, name="Oc")
    nc.scalar.copy(out=Oc, in_=oc_ps)
    yield

    # rb (+c_send), replicated 4x along h for the batched v matmul
    rb_ps = ps.tile([128, DC, G, R], FP, name="rb_ps", tag="big", bufs=3)
    for dc in range(DC):
        for k in range(DC):
            nc.tensor.matmul(out=rb_ps[:, dc, :, :],
                             lhsT=w["Wo_s"][:, k, dc * 128:(dc + 1) * 128],
                             rhs=Oc[:, k, :, :],
                             start=(k == 0), stop=(k == DC - 1))
    rb4 = psm.tile([128, DC, H, G, R], BF, name="rb4")
    csx = w["c_sendX"].rearrange("p c x -> p (c x)") \
        .rearrange("p (c g r) -> p c g r", c=DC, g=G)
    for h in range(H):
        nc.vector.tensor_add(out=rb4[:, :, h, :, :], in0=rb_ps, in1=csx)
    yield

    # receiver k (+bk_r)
    kt_ps = ps.tile([128, DC, G, R], FP, name="kt_ps", tag="big", bufs=3)
    for dc in range(DC):
        for k in range(DC):
            nc.tensor.matmul(out=kt_ps[:, dc, :, :],
                             lhsT=w["Wk_r"][:, k, dc * 128:(dc + 1) * 128],
                             rhs=rb4[:, k, 0, :, :],
                             start=(k == 0), stop=(k == DC - 1))
    kT = psm.tile([128, DC, G, R], BF, name="kT")
    nc.vector.tensor_add(out=kT, in0=kt_ps,
                         in1=w["bk_rX"].rearrange("p c x -> p (c x)")
                         .rearrange("p (c g r) -> p c g r", c=DC, g=G))
    yield

    # Wtil = Wq_r-fold through k; rank-1 bias row
    wt_ps = ps.tile([128, DC, H, G * R], FP, name="wt_ps", tag="big", bufs=3)
    for dc in range(DC):
        for h in range(H):
            nc.tensor.matmul(out=wt_ps[:, dc, h, :],
                             lhsT=w["WqrT"][:, h, dc * 128:(dc + 1) * 128],
                             rhs=kT[:, h, :, :], start=True, stop=True)
    Wtil = psm.tile([128, DC, H * G * R], BF, name="Wtil")
    nc.scalar.copy(out=Wtil, in_=wt_ps.rearrange("p c h x -> p c (h x)"))
    br_ps = ps.tile([1, H, G * R], FP, name="br_ps", tag="tp", bufs=2)
    for h in range(H):
        nc.tensor.matmul(out=br_ps[:, h, :],
                         lhsT=w["bqr_e"][:, h:h + 1],
                         rhs=kT[:, h, :, :], start=True, stop=True)
    brow = psm.tile([1, H * G * R], BF, name="brow")
    nc.scalar.copy(out=brow, in_=br_ps.rearrange("p h x -> p (h x)"))
    yield

    # receiver scores + exp
    s2_ps = ps.tile([128, T], FP, name="s2_ps", tag="big", bufs=3)
    for k in range(DC):
        nc.tensor.matmul(out=s2_ps, lhsT=Wtil[:, k, :], rhs=xT[:, k, :],
                         start=(k == 0), stop=False)
    nc.tensor.matmul(out=s2_ps, lhsT=brow, rhs=cst["ones_rowb"],
                     start=False, stop=False)
    nc.tensor.matmul(out=s2_ps, lhsT=w["U4"], rhs=w["V4"],
                     start=False, stop=True)
    e2a = psm.tile([64, T], BF, name="e2a")
    nc.scalar.activation(out=e2a, in_=s2_ps[0:64, :], func=ACTF.Exp)
    e2b = psm.tile([64, T], BF, name="e2b")
    nc.scalar.activation(out=e2b, in_=s2_ps[64:128, :], func=ACTF.Exp)
    yield

    # denominators + reciprocals; batched v
    r2h = []
    for h in range(H):
        base = (h % 2) * HR
        den_h = ps.tile([1, T], FP, name=f"den{h}", tag="tp", bufs=2)
        nc.tensor.matmul(out=den_h, lhsT=onesb[base:base + HR, :],
                         rhs=[e2a, e2b][h // 2][base:base + HR, :],
                         start=True, stop=True)
        rh = psm.tile([1, T], FP, name=f"r2_{h}", tag="r2h", bufs=4)
        nc.vector.reciprocal_approx_fast(out=rh, in_=den_h)
        r2h.append(rh)
    v_ps = ps.tile([128, D], FP, name="v_ps", tag="big", bufs=3)
    for k in range(DC):
        nc.tensor.matmul(out=v_ps,
                         lhsT=rb4[:, k, :, :, :].rearrange("p h g r -> p (h g r)"),
                         rhs=w["Wv_r"][:, k, :],
                         start=(k == 0), stop=(k == DC - 1))
    v_sb = psm.tile([128, D], BF, name="v_sb")
    nc.scalar.copy(out=v_sb, in_=v_ps)
    yield

    # mix + normalize
    aT = pa.tile([128, H, T], BF, name="aT")
    e2ab = [e2a, e2b]
    for h in range(H):
        recB = pb.tile([128, T], FP, name=f"recB{h}", tag="recB", bufs=2)
        nc.gpsimd.partition_broadcast(recB, r2h[h])
        base = (h % 2) * HR
        mx_ps = ps.tile([128, T], FP, name="mx_ps", tag="big", bufs=3)
        nc.tensor.matmul(out=mx_ps,
                         lhsT=v_sb[base:base + HR, h * E:(h + 1) * E],
                         rhs=e2ab[h // 2][base:base + HR, :],
                         start=True, stop=True)
        nc.vector.tensor_mul(out=aT[:, h, :], in0=mx_ps, in1=recB)
        if h == 1:
            yield
    yield

    # attn2 + residual 1
    s1T = pa.tile([128, DC, T], BF, name="s1T")
    for dc in range(DC):
        at_ps = ps.tile([128, T], FP, name="at_ps", tag="big", bufs=3)
        for k in range(DC):
            nc.tensor.matmul(out=at_ps,
                             lhsT=w["Wo_r"][:, k, dc * 128:(dc + 1) * 128],
                             rhs=aT[:, k, :], start=(k == 0), stop=(k == DC - 1))
        nc.vector.scalar_tensor_tensor(out=s1T[:, dc, :],
                                       in0=at_ps,
                                       scalar=w["c_recv"][:, dc:dc + 1],
                                       in1=xT[:, dc, :],
                                       op0=OP.add, op1=OP.add)
        if dc == 1:
            yield
    yield

    # LN1 statistics
    mean_ps = ps.tile([1, T], FP, name="mean_ps", tag="tp", bufs=2)
    for k in range(DC):
        nc.tensor.matmul(out=mean_ps, lhsT=onesb, rhs=s1T[:, k, :],
                         start=(k == 0), stop=(k == DC - 1))
    msc = psm.tile([1, T], FP, name="msc")
    nc.scalar.activation(out=msc, in_=mean_ps, func=ACTF.Copy, scale=1.0 / D)
    sqt = pb.tile([128, DC, T], BF, name="sqt", tag="sqt", bufs=2)
    nc.vector.tensor_mul(out=sqt.rearrange("p c t -> p (c t)"),
                         in0=s1T.rearrange("p c t -> p (c t)"),
                         in1=s1T.rearrange("p c t -> p (c t)"))
    ss_ps = ps.tile([1, T], FP, name="ss_ps", tag="tp", bufs=2)
    for k in range(DC):
        nc.tensor.matmul(out=ss_ps, lhsT=onesb, rhs=sqt[:, k, :],
                         start=(k == 0), stop=(k == DC - 1))
    msc2 = psm.tile([1, T], FP, name="msc2")
    nc.vector.tensor_mul(out=msc2, in0=msc, in1=msc)
    var_s = psm.tile([1, T], FP, name="var_s")
    nc.vector.scalar_tensor_tensor(out=var_s, in0=ss_ps, scalar=1.0 / D,
                                   in1=msc2, op0=OP.mult, op1=OP.subtract)
    srt = psm.tile([1, T], FP, name="srt")
    nc.scalar.activation(out=srt, in_=var_s, func=ACTF.Sqrt, bias=cst["eps_t"])
    rstd = psm.tile([1, T], FP, name="rstd")
    nc.vector.reciprocal_approx_fast(out=rstd, in_=srt)
    yield

    # LN1 normalize (raw: gains folded downstream)
    rstdB = pb.tile([128, T], FP, name="rstdB", tag="rstdB", bufs=2)
    nc.gpsimd.partition_broadcast(rstdB, rstd)
    mscB = pb.tile([128, T], FP, name="mscB", tag="mscB", bufs=2)
    nc.gpsimd.partition_broadcast(mscB, msc)
    out1T = pa.tile([128, DC, T], BF, name="out1T")
    st["out1T"] = out1T
    for dc in range(DC):
        t1 = pb.tile([128, T], FP, name="t1", tag="t1", bufs=2)
        nc.vector.tensor_sub(out=t1, in0=s1T[:, dc, :], in1=mscB)
        nc.vector.tensor_mul(out=out1T[:, dc, :], in0=t1, in1=rstdB)
        if dc == 1:
            yield
    yield


def gen_B(nc, w, cst, pools, out, gi, st):
    """Back-end: MLP (h2 token-oriented, b2+out1 folded into the PSUM
    accumulation), LN2 over the free dim, store."""
    pa, psm, pb, ps = (pools[k] for k in ("pa", "psm", "pb", "ps"))
    r0 = gi * T
    out1T = st["out1T"]

    h2_ps = [ps.tile([128, D], FP, name=f"h2_ps{a}", tag=f"h2_{a}", bufs=1)
             for a in range(TA)]
    for a in range(TA):
        nc.tensor.matmul(out=h2_ps[a], lhsT=cst["onescol_b"], rhs=w["b2row"],
                         start=True, stop=False)
    yield

    for oc in range(OC):
        h1_ps = ps.tile([128, T], FP, name="h1_ps", tag="big", bufs=3)
        for k in range(DC):
            nc.tensor.matmul(out=h1_ps,
                             lhsT=w["W1"][:, k, oc * 128:(oc + 1) * 128],
                             rhs=out1T[:, k, :], start=(k == 0), stop=(k == DC - 1))
        gl = pb.tile([128, T], BF, name="gl", tag="gl", bufs=3)
        nc.scalar.activation(out=gl, in_=h1_ps, func=ACTF.Gelu,
                             bias=w["b1"][:, oc:oc + 1])
        for a in range(TA):
            nc.tensor.matmul(out=h2_ps[a],
                             lhsT=gl[:, a * 128:(a + 1) * 128],
                             rhs=w["W2"][:, oc, :],
                             start=False, stop=False)
        yield

    # residual: out1 (token rows, ln1_g-scaled) accumulated into h2 PSUM
    for a in range(TA):
        for dc in range(DC):
            nc.tensor.matmul(out=h2_ps[a][:, dc * 128:(dc + 1) * 128],
                             lhsT=out1T[:, dc, a * 128:(a + 1) * 128],
                             rhs=w["Idg1"][:, dc, :],
                             start=False, stop=(dc == DC - 1))
        yield

    # LN2 statistics per token chunk (copy h2 PSUM to SBUF, then stats)
    m2t = psm.tile([128, TA], FP, name="m2t")
    var2 = psm.tile([128, TA], FP, name="var2")
    s2t = pb.tile([128, TA, D], FP, name="s2t", tag="s2t", bufs=2)
    for a in range(TA):
        nc.vector.tensor_scalar_mul(out=s2t[:, a, :], in0=h2_ps[a],
                                    scalar1=1.0)
        sum2 = psm.tile([128, 1], FP, name=f"sum2_{a}", tag="sum2", bufs=2)
        nc.vector.tensor_reduce(out=sum2, in_=s2t[:, a, :], axis=AX.X,
                                op=OP.add)
        nc.vector.tensor_scalar_mul(out=m2t[:, a:a + 1], in0=sum2,
                                    scalar1=1.0 / D)
        sq2 = pb.tile([128, D], BF, name="sq2", tag="sq2", bufs=2)
        nc.vector.tensor_mul(out=sq2, in0=s2t[:, a, :], in1=s2t[:, a, :])
        ssum2 = psm.tile([128, 1], FP, name=f"ssum2_{a}", tag="ssum2", bufs=2)
        nc.vector.tensor_reduce(out=ssum2, in_=sq2, axis=AX.X, op=OP.add)
        mm2 = psm.tile([128, 1], FP, name=f"mm2_{a}", tag="mm2", bufs=2)
        nc.vector.tensor_mul(out=mm2, in0=m2t[:, a:a + 1], in1=m2t[:, a:a + 1])
        nc.vector.scalar_tensor_tensor(out=var2[:, a:a + 1], in0=ssum2,
                                       scalar=1.0 / D, in1=mm2,
                                       op0=OP.mult, op1=OP.subtract)
        yield

    srt2 = psm.tile([128, TA], FP, name="srt2")
    nc.scalar.activation(out=srt2, in_=var2, func=ACTF.Sqrt,
                         bias=cst["eps_col"])
    rstd2 = psm.tile([128, TA], FP, name="rstd2")
    nc.vector.reciprocal_approx_fast(out=rstd2, in_=srt2)
    yield

    out_tok = pa.tile([128, TA, D], FP, name="out_tok")
    for a in range(TA):
        xc = pb.tile([128, D], FP, name="xc", tag="xc", bufs=2)
        nc.vector.tensor_scalar(out=xc, in0=s2t[:, a, :],
                                scalar1=m2t[:, a:a + 1], op0=OP.subtract,
                                scalar2=rstd2[:, a:a + 1], op1=OP.mult)
        gx = pb.tile([128, D], FP, name="gx", tag="gx", bufs=2)
        nc.gpsimd.tensor_mul(out=gx, in0=xc, in1=w["g2B"])
        nc.vector.tensor_add(out=out_tok[:, a, :], in0=gx, in1=w["b2bB"])
        yield
    nc.gpsimd.dma_start(out=out[r0:r0 + T, :].rearrange("(a p) d -> p a d", p=128),
                        in_=out_tok)
    yield


def _host_fold(inputs):
    """Host-side weight-only precomputation (bf16 for matmul operands)."""
    f32 = np.float32
    bf = ml_dtypes.bfloat16
    scale = 1.0 / np.sqrt(np.float32(E))

    def chunked(a):
        # [D_in, X] -> [128, DC_in * X] partition-major chunk layout
        d_in, x = a.shape
        c = d_in // 128
        return np.ascontiguousarray(
            a.reshape(c, 128, x).transpose(1, 0, 2).reshape(128, c * x))

    q_s = (inputs["router"] @ inputs["Wq_s"] + inputs["bq_s"]).astype(f32)
    q_sh = q_s.reshape(R, H, E)
    Wk = inputs["Wk_s"].reshape(D, H, E)
    M_score = (np.einsum("dhe,rhe->dhr", Wk, q_sh).reshape(D, HR) * scale).astype(f32)
    c_score = (np.einsum("he,rhe->hr", inputs["bk_s"].reshape(H, E), q_sh)
               .reshape(HR) * scale).astype(f32)
    c_send = (inputs["bv_s"] @ inputs["Wo_s"] + inputs["bo_s"]).astype(f32)
    c_recv = (inputs["bv_r"] @ inputs["Wo_r"] + inputs["bo_r"]).astype(f32)

    WqrT = (inputs["Wq_r"].astype(f32) * scale).reshape(D, H, E).transpose(2, 1, 0)
    WqrT = np.ascontiguousarray(WqrT.reshape(128, H * D))
    bqr_e = np.ascontiguousarray(
        (inputs["bq_r"].astype(f32) * scale).reshape(H, E).T)

    W1p = inputs["ln1_g"][:, None].astype(f32) * inputs["W1"].astype(f32)
    b1p = (inputs["b1"].astype(f32)
           + inputs["ln1_b"].astype(f32) @ inputs["W1"].astype(f32))
    b2row = (inputs["b2"].astype(f32) + inputs["ln1_b"].astype(f32))

    U4 = np.zeros((G, 128), f32)
    V4 = np.zeros((G, T), f32)
    for g in range(G):
        for p in range(128):
            if (p % HR) // R == g:
                U4[g, p] = 1.0
        V4[g, :] = -100.0
        V4[g, g * L:(g + 1) * L] = 0.0
    Msum = np.zeros((64, 2), f32)
    for p in range(64):
        Msum[p, p // HR] = 1.0

    g1 = inputs["ln1_g"].astype(f32)
    Idg1 = np.zeros((128, DC, 128), f32)
    for dc in range(DC):
        Idg1[:, dc, :] = np.diag(g1[dc * 128:(dc + 1) * 128])
    Idg1 = Idg1.reshape(128, DC * 128)

    def colvec(v):  # [D] -> [128, DC] (partition p, chunk c) = v[c*128+p]
        return np.ascontiguousarray(v.reshape(DC, 128).T.astype(f32))

    return {
        "Msc": chunked(M_score).astype(bf),
        "c_score": c_score.reshape(HR, 1),
        "c_send": colvec(c_send),
        "c_recv": colvec(c_recv),
        "bk_r": colvec(inputs["bk_r"].astype(f32)),
        "Wv_s": chunked(inputs["Wv_s"].astype(f32)).astype(bf),
        "Wo_s": chunked(inputs["Wo_s"].astype(f32)).astype(bf),
        "Wk_r": chunked(inputs["Wk_r"].astype(f32)).astype(bf),
        "Wv_r": chunked(inputs["Wv_r"].astype(f32)).astype(bf),
        "Wo_r": chunked(inputs["Wo_r"].astype(f32)).astype(bf),
        "WqrT": WqrT.astype(bf),
        "bqr_e": bqr_e.astype(bf),
        "W1": chunked(W1p).astype(bf),
        "b1": np.ascontiguousarray(b1p.reshape(OC, 128).T),
        "W2": chunked(inputs["W2"].astype(f32)).astype(bf),
        "b2row": b2row.reshape(1, D).astype(bf),
        "U4": U4.astype(bf),
        "V4": V4.astype(bf),
        "Msum": Msum.astype(bf),
        "Idg1": Idg1.astype(bf),
        "g2row": inputs["ln2_g"].astype(f32).reshape(1, D),
        "b2brow": inputs["ln2_b"].astype(f32).reshape(1, D),
    }


def _core_in_maps(Z, folded):
    """Per-core input maps (Z full fp32 array [B, C, L, D])."""
    n_seq_total = B * C
    n_seq = n_seq_total // N_CORES
    Zb = Z.reshape(n_seq_total, L, D).astype(ml_dtypes.bfloat16)
    in_maps = []
    for c in range(N_CORES):
        m = {"z": np.ascontiguousarray(
            Zb[c * n_seq:(c + 1) * n_seq].reshape(n_seq * L, D))}
        m.update(folded)
        in_maps.append(m)
    return in_maps


def kernel(**inputs) -> np.ndarray:
    inputs = {k: np.asarray(v) for k, v in inputs.items()}
    Z = inputs["Z"].astype(np.float32)
    n_seq_total = B * C
    n_seq = n_seq_total // N_CORES
    folded = _host_fold(inputs)

    nc = build_core_kernel(n_seq)
    in_maps = _core_in_maps(Z, folded)
    res = run_bass_kernel_spmd(nc, in_maps, list(range(N_CORES)))
    out = np.empty((n_seq_total, L, D), np.float32)
    for c in range(N_CORES):
        out[c * n_seq:(c + 1) * n_seq] = res.results[c]["out"].reshape(n_seq, L, D)
    return out.reshape(B, C, L, D)


if __name__ == "__main__":
    import reference
    inputs = reference.setup_inputs()
    inputs = {k: np.asarray(v) for k, v in inputs.items()}
    expected = np.asarray(reference.reference(**inputs))
    got = kernel(**inputs)
    err = np.abs(got - expected).max()
    rel = err / np.abs(expected).max()
    print(f"abs err {err:.3e}  absmax-rel {rel:.3e}")
